# revision 5
# baseline (speedup 1.0000x reference)
"""Trainium2 Bass kernel for a gated-attention MIL pooling model.

Model (per batch b of B=4, over n=10000 tiles, d=2048):
    M1 = relu(M @ Wr + br)                                (n, 512)
    w  = (tanh(M1@U) * sigmoid(M1@V)) @ Ww                (n, 1)
    a  = softmax(w, axis over n)
    r  = sum_n a * M1                                     (512,)
    slide_out = sigmoid(relu(r@W1+b1)@W2+b2)              scalar
    tiles_out = sigmoid(M1@Wt + bt)                       (n, 1)

Distribution: 8 NeuronCores, core c handles batch c//2, token half c%2
(5000 tokens, zero-padded to 5120).  Each core returns raw attention
scores w, tile logits, P = sum_t exp(w_t) * M1_t and S = sum_t exp(w_t);
the softmax normalization, the tiny slide classifier head and the final
sigmoids run on host in fp32.
"""

import numpy as np
from contextlib import ExitStack

import concourse.bass as bass
import concourse.bacc as bacc
import concourse.tile as tile
from concourse import mybir
from concourse.bass_utils import run_bass_kernel_spmd
from concourse.masks import make_identity

F32 = mybir.dt.float32
BF16 = mybir.dt.bfloat16
AX = mybir.AxisListType
AF = mybir.ActivationFunctionType

B, N, D, D1, D2, A = 4, 10000, 2048, 512, 256, 128
NSH = N // 2          # tokens per core
T = 512               # tokens per group
G = (NSH + T - 1) // T  # 10 groups
NPAD = G * T          # 5120
KC = D // 128         # 16 k-chunks
FC = D1 // 128        # 4 f1-chunks
TC = T // 128         # 4 token-chunks per group

_nc_cache = None


def _build_nc(reps=1):
    nc = bacc.Bacc(None, target_bir_lowering=False)

    Msh = nc.dram_tensor("Msh", [NPAD, D], F32, kind="ExternalInput")
    Wr = nc.dram_tensor("Wr", [D, D1], F32, kind="ExternalInput")
    br = nc.dram_tensor("br", [D1], F32, kind="ExternalInput")
    U = nc.dram_tensor("U", [D1, A], F32, kind="ExternalInput")
    V = nc.dram_tensor("V", [D1, A], F32, kind="ExternalInput")
    Ww = nc.dram_tensor("Ww", [A, 1], F32, kind="ExternalInput")
    Wt = nc.dram_tensor("Wt", [D1, 1], F32, kind="ExternalInput")

    w_out = nc.dram_tensor("w_out", [1, NPAD], F32, kind="ExternalOutput")
    tl_out = nc.dram_tensor("tl_out", [1, NPAD], F32, kind="ExternalOutput")
    P_out = nc.dram_tensor("P_out", [1, D1], F32, kind="ExternalOutput")
    S_out = nc.dram_tensor("S_out", [1, 1], F32, kind="ExternalOutput")

    with tile.TileContext(nc) as tc, ExitStack() as ctx:
        singles = ctx.enter_context(tc.tile_pool(name="singles", bufs=1))
        mnat_p = ctx.enter_context(tc.tile_pool(name="mnat", bufs=2))
        mt_p = ctx.enter_context(tc.tile_pool(name="mt", bufs=2))
        m1t_p = ctx.enter_context(tc.tile_pool(name="m1t", bufs=2))
        m1n_p = ctx.enter_context(tc.tile_pool(name="m1n", bufs=3))
        gate_p = ctx.enter_context(tc.tile_pool(name="gate", bufs=2))

        pp_t = ctx.enter_context(tc.tile_pool(name="pp_t", bufs=3, space="PSUM"))
        pp_main = ctx.enter_context(tc.tile_pool(name="pp_main", bufs=2, space="PSUM"))
        pp_misc = ctx.enter_context(tc.tile_pool(name="pp_misc", bufs=2, space="PSUM"))
        pp_P = ctx.enter_context(tc.tile_pool(name="pp_P", bufs=1, space="PSUM"))

        # ---- one-time weight staging (bf16 casts during DMA, SWDGE) ----
        Wr_sb = singles.tile([128, KC, D1], BF16)
        nc.gpsimd.dma_start(out=Wr_sb, in_=Wr.rearrange("(kc p) f -> p kc f", p=128))
        U_sb = singles.tile([128, FC, A], BF16)
        nc.gpsimd.dma_start(out=U_sb, in_=U.rearrange("(c p) a -> p c a", p=128))
        V_sb = singles.tile([128, FC, A], BF16)
        nc.gpsimd.dma_start(out=V_sb, in_=V.rearrange("(c p) a -> p c a", p=128))
        Ww_sb = singles.tile([128, 1], BF16)
        nc.gpsimd.dma_start(out=Ww_sb, in_=Ww[:, :])
        Wt_sb = singles.tile([128, FC], BF16)
        nc.gpsimd.dma_start(out=Wt_sb, in_=Wt.rearrange("(c p) one -> p (c one)", p=128))
        br_sb = singles.tile([128, FC], F32)
        nc.gpsimd.dma_start(out=br_sb, in_=br.rearrange("(c p) -> p c", p=128))

        ident = singles.tile([128, 128], BF16)
        make_identity(nc, ident)
        ones_sb = singles.tile([1, 1], F32)
        nc.vector.memset(ones_sb, 1.0)

        e_row = singles.tile([1, NPAD], F32)
        w_row = singles.tile([1, NPAD], F32)
        tl_row = singles.tile([1, NPAD], F32)
        eT_sb = singles.tile([128, G * TC], BF16)
        acc_sb = singles.tile([1, G], F32)
        P_sb = singles.tile([1, D1], F32)
        S_sb = singles.tile([1, 1], F32)

        P_ps = pp_P.tile([1, D1], F32)

        mnats = [None] * G
        mts = [None] * G

        def phase_load(g):
            """DMA load + PE transpose M -> MT for group g."""
            mnat = mnat_p.tile([128, TC, D], BF16, tag="mnat")
            mnats[g] = mnat
            for t in range(TC):
                nc.gpsimd.dma_start(
                    out=mnat[:, t, :],
                    in_=Msh[g * T + t * 128 : g * T + (t + 1) * 128, :],
                )
            mt = mt_p.tile([128, KC, T], BF16, tag="mt")
            mts[g] = mt
            for kc in range(KC):
                ps = pp_t.tile([128, T], BF16, tag="tp")
                for t in range(TC):
                    nc.tensor.transpose(
                        ps[:, t * 128 : (t + 1) * 128],
                        mnat[:, t, kc * 128 : (kc + 1) * 128],
                        ident,
                    )
                # alternate copy engine to balance ACT/DVE
                if kc % 2 == 0:
                    nc.scalar.copy(mt[:, kc, :], ps)
                else:
                    nc.vector.tensor_copy(mt[:, kc, :], ps)

        def phase_compute(g):
            mt = mts[g]
            sl = slice(g * T, (g + 1) * T)

            # ---- M1T = relu(Wr.T @ MT + br) ----
            m1t = m1t_p.tile([128, FC, T], BF16, tag="m1t")
            for fc in range(FC):
                ps = pp_main.tile([128, T], F32, tag="main")
                for kc in range(KC):
                    nc.tensor.matmul(
                        ps,
                        Wr_sb[:, kc, fc * 128 : (fc + 1) * 128],
                        mt[:, kc, :],
                        start=(kc == 0),
                        stop=(kc == KC - 1),
                    )
                nc.scalar.activation(
                    out=m1t[:, fc, :], in_=ps, func=AF.Relu,
                    bias=br_sb[:, fc : fc + 1], scale=1.0,
                )

            # ---- gated attention scores ----
            ps_u = pp_misc.tile([128, T], F32, tag="misc")
            for fc in range(FC):
                nc.tensor.matmul(
                    ps_u, U_sb[:, fc, :], m1t[:, fc, :],
                    start=(fc == 0), stop=(fc == FC - 1),
                )
            t_sb = gate_p.tile([128, T], BF16, tag="tanh")
            nc.scalar.activation(out=t_sb, in_=ps_u, func=AF.Tanh)

            ps_v = pp_misc.tile([128, T], F32, tag="misc")
            for fc in range(FC):
                nc.tensor.matmul(
                    ps_v, V_sb[:, fc, :], m1t[:, fc, :],
                    start=(fc == 0), stop=(fc == FC - 1),
                )
            s_sb = gate_p.tile([128, T], BF16, tag="sig")
            # sigmoid(x) = 0.5*tanh(x/2) + 0.5  (stays on the exp/tanh table set)
            nc.scalar.activation(out=s_sb, in_=ps_v, func=AF.Tanh, scale=0.5)
            g_sb = gate_p.tile([128, T], BF16, tag="gmul")
            nc.vector.tensor_scalar(
                out=s_sb, in0=s_sb, scalar1=0.5, scalar2=0.5,
                op0=mybir.AluOpType.mult, op1=mybir.AluOpType.add,
            )
            nc.vector.tensor_mul(g_sb, t_sb, s_sb)

            # ---- w scores + exp ----
            ps_w = pp_misc.tile([1, T], F32, tag="misc")
            nc.tensor.matmul(ps_w, Ww_sb, g_sb, start=True, stop=True)
            nc.vector.tensor_copy(w_row[0:1, sl], ps_w)
            if g < G - 1:
                nc.scalar.activation(
                    out=e_row[0:1, sl], in_=ps_w, func=AF.Exp,
                    accum_out=acc_sb[0:1, g : g + 1],
                )
            else:
                nc.scalar.activation(out=e_row[0:1, sl], in_=ps_w, func=AF.Exp)
                # zero the padded tail, then sum the valid remainder
                nc.vector.memset(e_row[0:1, NSH:NPAD], 0.0)
                nc.vector.reduce_sum(
                    out=acc_sb[0:1, g : g + 1],
                    in_=e_row[0:1, g * T : NPAD],
                    axis=AX.X,
                )

            # ---- tiles logits ----
            ps_tl = pp_misc.tile([1, T], F32, tag="misc")
            for fc in range(FC):
                nc.tensor.matmul(
                    ps_tl, Wt_sb[:, fc : fc + 1], m1t[:, fc, :],
                    start=(fc == 0), stop=(fc == FC - 1),
                )
            nc.vector.tensor_copy(tl_row[0:1, sl], ps_tl)

            # ---- e^T chunks (128,1) via ones-trick ----
            ps_eT = pp_misc.tile([128, TC], F32, tag="misc")
            for t in range(TC):
                c = g * TC + t
                nc.tensor.matmul(
                    ps_eT[:, t : t + 1],
                    e_row[0:1, c * 128 : (c + 1) * 128],
                    ones_sb,
                    start=True, stop=True,
                )
            nc.vector.tensor_copy(eT_sb[:, g * TC : (g + 1) * TC], ps_eT)

            # ---- M1 natural (token-major) + P accumulation ----
            for t in range(TC):
                c = g * TC + t
                ps_n = pp_t.tile([128, D1], BF16, tag="tp")
                for fc in range(FC):
                    nc.tensor.transpose(
                        ps_n[:, fc * 128 : (fc + 1) * 128],
                        m1t[:, fc, t * 128 : (t + 1) * 128],
                        ident,
                    )
                m1n = m1n_p.tile([128, D1], BF16, tag="m1n")
                nc.scalar.copy(m1n, ps_n)
                nc.tensor.matmul(
                    P_ps,
                    eT_sb[:, c : c + 1],
                    m1n,
                    start=(c == 0),
                    stop=(c == G * TC - 1),
                )
            mnats[g] = None
            mts[g] = None

        # 1-stage software pipeline: load(g+1) overlaps compute(g)
        # reps>1 repeats the whole body (idempotent) for timing runs
        for _rep in range(reps):
            phase_load(0)
            for g in range(G):
                if g + 1 < G:
                    phase_load(g + 1)
                phase_compute(g)

        # ---- epilogue: S, P, DMA out ----
        nc.vector.reduce_sum(out=S_sb, in_=acc_sb, axis=AX.X)
        nc.vector.tensor_copy(P_sb, P_ps)
        nc.sync.dma_start(out=w_out[:, :], in_=w_row)
        nc.sync.dma_start(out=tl_out[:, :], in_=tl_row)
        nc.sync.dma_start(out=P_out[:, :], in_=P_sb)
        nc.sync.dma_start(out=S_out[:, :], in_=S_sb)

    nc.compile()
    return nc


def _get_nc():
    global _nc_cache
    if _nc_cache is None:
        _nc_cache = _build_nc()
    return _nc_cache


def _sigmoid(x):
    return 1.0 / (1.0 + np.exp(-x))


def kernel(**inputs):
    M = np.ascontiguousarray(np.asarray(inputs["M"], dtype=np.float32))
    Wr = np.ascontiguousarray(np.asarray(inputs["Wr"], dtype=np.float32))
    br = np.ascontiguousarray(np.asarray(inputs["br"], dtype=np.float32))
    U = np.ascontiguousarray(np.asarray(inputs["U"], dtype=np.float32))
    V = np.ascontiguousarray(np.asarray(inputs["V"], dtype=np.float32))
    Ww = np.ascontiguousarray(np.asarray(inputs["Ww"], dtype=np.float32))
    W1 = np.asarray(inputs["W1"], dtype=np.float32)
    b1 = np.asarray(inputs["b1"], dtype=np.float32)
    W2 = np.asarray(inputs["W2"], dtype=np.float32)
    b2 = np.asarray(inputs["b2"], dtype=np.float32)
    Wt = np.ascontiguousarray(np.asarray(inputs["Wt"], dtype=np.float32))
    bt = np.asarray(inputs["bt"], dtype=np.float32)

    nc = _get_nc()

    in_maps = []
    for c in range(8):
        b, h = divmod(c, 2)
        msh = np.zeros((NPAD, D), dtype=np.float32)
        msh[:NSH] = M[b, h * NSH : (h + 1) * NSH, :]
        in_maps.append(
            {"Msh": msh, "Wr": Wr, "br": br, "U": U, "V": V, "Ww": Ww, "Wt": Wt}
        )

    res = run_bass_kernel_spmd(nc, in_maps, core_ids=list(range(8))).results

    slide_out = np.zeros((B,), dtype=np.float32)
    tiles_out = np.zeros((B, N, 1), dtype=np.float32)
    a_out = np.zeros((B, N), dtype=np.float32)
    for b in range(B):
        r0, r1 = res[2 * b], res[2 * b + 1]
        w_full = np.concatenate([r0["w_out"][0, :NSH], r1["w_out"][0, :NSH]])
        m = w_full.max()
        e = np.exp(w_full - m)
        a_out[b] = e / e.sum()
        r = (r0["P_out"][0] + r1["P_out"][0]) / (
            r0["S_out"][0, 0] + r1["S_out"][0, 0]
        )
        h1 = np.maximum(r @ W1 + b1, 0.0)
        slide_out[b] = _sigmoid(h1 @ W2 + b2)[0]
        tl = np.concatenate([r0["tl_out"][0, :NSH], r1["tl_out"][0, :NSH]])
        tiles_out[b, :, 0] = _sigmoid(tl + bt[0])

    return slide_out, tiles_out, a_out
